# revision 12
# baseline (speedup 1.0000x reference)
import sys

if "/opt/trn_rl_repo" not in sys.path:
    sys.path.insert(0, "/opt/trn_rl_repo")

import heapq
import itertools

import numpy as np

import concourse.bacc as bacc
import concourse.tile as tile
from concourse import bass_utils, mybir
from concourse.bass import ts

F32 = mybir.dt.float32
BF16 = mybir.dt.bfloat16
EXP = mybir.ActivationFunctionType.Exp


# nn_MultiHeadedAttention: B=2, S=2048, D=1024, H=16, DH=64.
# 16 heads over 8 cores (2 heads/core = 128 features). QKV column-parallel,
# out-projection row-parallel, host sums the 8 partial outputs.
#
# K-bias is dropped entirely: softmax(q.(k+bk)) == softmax(q.k + const_per_q).
# V-bias folds through softmax on the host (rows of p sum to 1).
# V transposed to [tok, vfeat] via XBAR DMA transpose (no PE transposes).
B, S, D, H = 2, 2048, 1024, 16
DH = D // H
NC = 8
T = B * S                  # 4096 tokens
NCHUNK = T // 512          # 8 token chunks of 512
KCH = D // 128             # 8 contraction chunks
NJ = S // 128              # 16 key tiles per batch
QC = S // 512              # 4 query chunks per batch
NTT = T // 128             # 32 token tiles of 128 (v.T groups)

_CACHE = {}


def _build():
    if "nc" in _CACHE:
        return _CACHE["nc"]

    nc = bacc.Bacc("TRN2", target_bir_lowering=False, debug=False,
                   enable_asserts=True, num_devices=NC)

    xT = nc.dram_tensor("xT", [D, T], BF16, kind="ExternalInput").ap()
    wq = nc.dram_tensor("wq", [D, 128], BF16, kind="ExternalInput").ap()
    wk = nc.dram_tensor("wk", [D, 128], BF16, kind="ExternalInput").ap()
    wv = nc.dram_tensor("wv", [D, 128], BF16, kind="ExternalInput").ap()
    wo = nc.dram_tensor("wo", [128, D], BF16, kind="ExternalInput").ap()
    bq = nc.dram_tensor("bq", [128, 1], F32, kind="ExternalInput").ap()
    outT = nc.dram_tensor("outT", [D, T], BF16, kind="ExternalOutput").ap()

    with tile.TileContext(nc) as tc:
        with (
            tc.tile_pool(name="wpool", bufs=1) as wpool,
            tc.tile_pool(name="xin", bufs=8) as xin_pool,
            tc.tile_pool(name="epool", bufs=4) as epool,
            tc.tile_pool(name="vstg", bufs=2) as vstg_pool,
            tc.tile_pool(name="vgt", bufs=3) as vgt_pool,
            tc.tile_pool(name="onp", bufs=3) as on_pool,
            tc.tile_pool(name="npool", bufs=2) as npool,
            tc.tile_pool(name="ostage", bufs=3) as ostage_pool,
            # PSUM: spool 2x[128,1024]f32 (scores), opool 1x[65,1024]
            # (o accum), ppool 2x[128,512] (projections / out-projection)
            tc.tile_pool(name="spool", bufs=2, space="PSUM") as spool,
            tc.tile_pool(name="opool", bufs=1, space="PSUM") as opool,
            tc.tile_pool(name="ppool", bufs=2, space="PSUM") as ppool,
        ):
            # ---- persistent weights / activations ----
            wq_sb = wpool.tile([128, D], BF16, name="wq_sb")
            wk_sb = wpool.tile([128, D], BF16, name="wk_sb")
            wv_sb = wpool.tile([128, D], BF16, name="wv_sb")
            wo_sb = wpool.tile([128, D], BF16, name="wo_sb")
            bq_sb = wpool.tile([128, 1], F32, name="bq_sb")
            nc.sync.dma_start(wq_sb[:], wq.rearrange("(k p) f -> p k f", p=128))
            nc.sync.dma_start(wk_sb[:], wk.rearrange("(k p) f -> p k f", p=128))
            nc.sync.dma_start(wv_sb[:], wv.rearrange("(k p) f -> p k f", p=128))
            nc.sync.dma_start(wo_sb[:], wo[:])
            nc.sync.dma_start(bq_sb[:], bq[:])

            qn_all = wpool.tile([128, T], BF16, name="qn_all")
            kn_all = wpool.tile([128, T], BF16, name="kn_all")
            on = [wpool.tile([128, 512], BF16, name=f"on{n}")
                  for n in range(NCHUNK)]
            # v.T groups: one per 128-token tile, [tok, h0(64)|1|h1(64)|1]
            vg = [wpool.tile([128, 130], BF16, name=f"vg{J}")
                  for J in range(NTT)]

            # input chunks: xp[c][:, k*512 + t] = x.T[k*128+p, c*512+t]
            xp = [xin_pool.tile([128, KCH * 512], BF16, tag="xp",
                                name=f"xp{c}") for c in range(NCHUNK)]

            def emit_xload(c, g):
                for k in range(4 * g, 4 * g + 4):
                    nc.sync.dma_start(xp[c][:, ts(k, 512)],
                                      xT[ts(k, 128), ts(c, 512)])

            for c in range(2):
                emit_xload(c, 0)
                emit_xload(c, 1)

            # Warm the ACT exp table while the prologue runs.
            dummy = wpool.tile([1, 2], F32, name="dummy")
            nc.vector.memset(dummy[:], 0.0)
            nc.scalar.activation(dummy[:], dummy[:], EXP)

            # ones columns of the v.T groups (cols 64 and 129), set once
            for J in range(NTT):
                nc.vector.memset(vg[J][:], 1.0)

            # ---- emission helpers ----
            proj_state = {}

            def emit_kproj(c, k0, k1):
                if k0 == 0:
                    proj_state[("k", c)] = ppool.tile([128, 512], F32,
                                                      tag="P", name=f"kps{c}")
                ps = proj_state[("k", c)]
                for k in range(k0, k1):
                    nc.tensor.matmul(ps[:, 0:512], wk_sb[:, ts(k, 128)],
                                     xp[c][:, ts(k, 512)], start=(k == 0),
                                     stop=(k == KCH - 1))
                if k1 == KCH:
                    nc.vector.tensor_copy(kn_all[:, ts(c, 512)], ps[:, 0:512])

            def emit_qproj(c, k0, k1):
                if k0 == 0:
                    proj_state[("q", c)] = ppool.tile([128, 512], F32,
                                                      tag="P", name=f"qps{c}")
                ps = proj_state[("q", c)]
                for k in range(k0, k1):
                    nc.tensor.matmul(ps[:, 0:512], wq_sb[:, ts(k, 128)],
                                     xp[c][:, ts(k, 512)], start=(k == 0),
                                     stop=(k == KCH - 1))
                if k1 == KCH:
                    nc.vector.tensor_scalar_add(qn_all[:, ts(c, 512)],
                                                ps[:, 0:512], bq_sb[:])

            def emit_vt(J):
                # v.T for token tile J directly: out[tok, vfeat] =
                # sum_k x_slice[feat,tok].T @ WvT[feat,vfeat]
                c, tt = J // 4, J % 4
                ps = ppool.tile([128, 512], F32, tag="P", name=f"vtps{J}")
                for k in range(KCH):
                    nc.tensor.matmul(
                        ps[:, 0:128],
                        xp[c][:, k * 512 + tt * 128: k * 512 + (tt + 1) * 128],
                        wv_sb[:, ts(k, 128)], start=(k == 0),
                        stop=(k == KCH - 1))
                nc.vector.tensor_copy(
                    vg[J].rearrange("p (h c2) -> p h c2", h=2)[:, :, 0:64],
                    ps[:, 0:128].rearrange("p (h c2) -> p h c2", h=2))

            def emit_norm(n, o_ps, use_scalar=False):
                # critical path first: sums -> recip -> broadcast -> mult
                sums_sb = npool.tile([1, 1024], F32, tag="sums",
                                     name=f"sums{n}")
                nc.vector.tensor_copy(sums_sb[:], o_ps[64:65, :])
                r_sb = npool.tile([1, 1024], F32, tag="r", name=f"r{n}")
                nc.vector.reciprocal_approx_fast(r_sb[:], sums_sb[0:1, :])
                oc = npool.tile([64, 1024], F32, tag="oc", name=f"oc{n}")
                if use_scalar:
                    nc.scalar.copy(oc[:], o_ps[0:64, :])
                else:
                    nc.vector.tensor_copy(oc[:], o_ps[0:64, :])
                for hh in range(2):
                    hs = slice(hh * 64, (hh + 1) * 64)
                    rb = npool.tile([64, 512], F32, tag=f"rb{hh}",
                                    name=f"rb{n}_{hh}")
                    nc.gpsimd.partition_broadcast(
                        rb[:], r_sb[0:1, ts(hh, 512)])
                    nc.vector.tensor_tensor(
                        out=on[n][hs, :], in0=oc[0:64, ts(hh, 512)],
                        in1=rb[:], op=mybir.AluOpType.mult)

            def emit_outproj_m(n, m, use_scalar=False):
                op_ps = ppool.tile([128, 512], F32, tag="P",
                                   name=f"opps{n}_{m}")
                nc.tensor.matmul(op_ps[:, 0:512], wo_sb[:, ts(m, 128)],
                                 on[n][:], start=True, stop=True)
                ost = ostage_pool.tile([128, 512], BF16, tag="ost",
                                       name=f"ost{n}_{m}")
                if use_scalar:
                    nc.scalar.copy(ost[:], op_ps[:, 0:512])
                else:
                    nc.vector.tensor_copy(ost[:], op_ps[:, 0:512])
                nc.sync.dma_start(outT[ts(m, 128), ts(n, 512)], ost[:])

            # ---- deferred-work heap: (deadline_slot, seq, cost_ns, fn) ----
            seq = itertools.count()
            work = []

            def push(deadline, cost, fn):
                heapq.heappush(work, (deadline, next(seq), cost, fn))

            BUDGET = 650.0

            def pop_work(slot):
                spent = 0.0
                while work and (work[0][0] <= slot + 1 or spent < BUDGET):
                    _, _, cost, fn = heapq.heappop(work)
                    fn()
                    spent += cost

            # ---- prologue: just enough for attention slot 0 ----
            emit_kproj(0, 0, KCH)
            emit_qproj(0, 0, KCH)
            emit_vt(0)
            emit_vt(1)

            # remaining inputs, staggered so they interleave on the queue
            for c in range(2, NCHUNK):
                dl = 2 * (c - 2) - 1
                push(dl, 100.0, (lambda c=c: emit_xload(c, 0)))
                push(dl, 100.0, (lambda c=c: emit_xload(c, 1)))
            # b0 keys: chunk c needed by scores slot 4c
            for c in range(1, QC):
                push(4 * c - 3, 3400.0, (lambda c=c: emit_kproj(c, 0, KCH)))
            # b1 keys: spread through b0's window
            for c in range(QC, NCHUNK):
                push(30 + 6 * (c - QC), 3400.0,
                     (lambda c=c: emit_kproj(c, 0, KCH)))
            # v.T tiles: b0 tile J by o-acc slot J+1, b1 spread early
            for J in range(2, NTT):
                if J < 16:
                    dl = J - 2
                else:
                    dl = 26 + 2 * (J - 16)
                push(dl, 1000.0, (lambda J=J: emit_vt(J)))
            # queries: chunk c by slot 16c
            for c in range(1, NCHUNK):
                push(16 * c - 5, 3400.0, (lambda c=c: emit_qproj(c, 0, KCH)))

            # ---- main attention loop ----
            for b in range(B):
                for qc in range(QC):
                    n = b * QC + qc
                    o_ps = opool.tile([65, 1024], F32, tag="O",
                                      name=f"ops{n}")
                    e_prev = None
                    for j in range(NJ):
                        slot = 16 * n + j
                        s_ps = spool.tile([128, 1024], F32, tag="S",
                                          name=f"sps{n}_{j}")
                        kblk = kn_all[:, b * S + j * 128: b * S + (j + 1) * 128]
                        for hh in range(2):
                            hs = slice(hh * 64, (hh + 1) * 64)
                            nc.tensor.matmul(
                                s_ps[:, ts(hh, 512)], kblk[hs, :],
                                qn_all[hs, ts(n, 512)], start=True, stop=True)
                        e_sb = epool.tile([128, 1024], BF16, tag="e",
                                          name=f"e{n}_{j}")
                        nc.scalar.activation(e_sb[:], s_ps[:], EXP)
                        if j >= 1:
                            for hh in range(2):
                                nc.tensor.matmul(
                                    o_ps[0:65, ts(hh, 512)],
                                    vg[b * NJ + j - 1][:,
                                                       hh * 65:(hh + 1) * 65],
                                    e_prev[:, ts(hh, 512)],
                                    start=(j - 1 == 0), stop=False)
                        e_prev = e_sb
                        pop_work(slot)
                    for hh in range(2):
                        nc.tensor.matmul(
                            o_ps[0:65, ts(hh, 512)],
                            vg[b * NJ + NJ - 1][:, hh * 65:(hh + 1) * 65],
                            e_prev[:, ts(hh, 512)], start=False, stop=True)
                    last = (n == NCHUNK - 1)
                    emit_norm(n, o_ps, use_scalar=last)
                    for m in range(KCH):
                        push(16 * (n + 1) + 4 + m, 400.0,
                             (lambda n=n, m=m, last=last:
                              emit_outproj_m(n, m, use_scalar=(last and
                                                               m % 2 == 1))))

            # drain whatever is left (last chunk's out-projection)
            while work:
                _, _, _, fn = heapq.heappop(work)
                fn()

    nc.compile()
    _CACHE["nc"] = nc
    return nc


def _prep_in_maps(inputs):
    import ml_dtypes
    bf16 = ml_dtypes.bfloat16
    x, Wq, bq = inputs["x"], inputs["Wq"], inputs["bq"]
    Wk, Wv, Wo = inputs["Wk"], inputs["Wv"], inputs["Wo"]
    xT = np.ascontiguousarray(x.reshape(T, D).T).astype(bf16)
    scale = np.float32(1.0 / np.sqrt(DH))
    in_maps = []
    for c in range(NC):
        sl = slice(128 * c, 128 * (c + 1))
        in_maps.append({
            "xT": xT,
            "wq": np.ascontiguousarray((scale * Wq[sl, :]).T).astype(bf16),
            "wk": np.ascontiguousarray(Wk[sl, :].T).astype(bf16),
            "wv": np.ascontiguousarray(Wv[sl, :].T).astype(bf16),
            "wo": np.ascontiguousarray(Wo[:, sl].T).astype(bf16),
            "bq": np.ascontiguousarray((scale * bq[sl])[:, None]),
        })
    return in_maps


def kernel(x, Wq, bq, Wk, bk, Wv, bv, Wo, bo):
    x = np.asarray(x, np.float32)
    Wq, bq = np.asarray(Wq, np.float32), np.asarray(bq, np.float32)
    Wk = np.asarray(Wk, np.float32)
    Wv, bv = np.asarray(Wv, np.float32), np.asarray(bv, np.float32)
    Wo, bo = np.asarray(Wo, np.float32), np.asarray(bo, np.float32)

    nc = _build()
    in_maps = _prep_in_maps({"x": x, "Wq": Wq, "bq": bq, "Wk": Wk,
                             "Wv": Wv, "Wo": Wo})
    res = bass_utils.run_bass_kernel_spmd(nc, in_maps, core_ids=list(range(NC)))

    acc = np.zeros((D, T), np.float64)
    for c in range(NC):
        acc += np.asarray(res.results[c]["outT"], np.float64)
    # v-bias folds through softmax (rows sum to 1): + bv @ Wo.T; plus bo.
    const = bo.astype(np.float64) + bv.astype(np.float64) @ Wo.T.astype(np.float64)
    out = acc.T + const[None, :]
    return out.astype(np.float32).reshape(B, S, D)


# revision 14
# speedup vs baseline: 1.1919x; 1.1919x over previous
import sys

if "/opt/trn_rl_repo" not in sys.path:
    sys.path.insert(0, "/opt/trn_rl_repo")

import heapq
import itertools

import numpy as np

import concourse.bacc as bacc
import concourse.tile as tile
from concourse import bass_utils, mybir
from concourse.bass import ts

F32 = mybir.dt.float32
BF16 = mybir.dt.bfloat16
EXP = mybir.ActivationFunctionType.Exp


# nn_MultiHeadedAttention: B=2, S=2048, D=1024, H=16, DH=64.
# 16 heads over 8 cores (2 heads/core = 128 features). QKV column-parallel,
# out-projection row-parallel, host sums the 8 partial outputs.
#
# K-bias is dropped entirely: softmax(q.(k+bk)) == softmax(q.k + const_per_q).
# V-bias folds through softmax on the host (rows of p sum to 1).
# V transposed to [tok, vfeat] via XBAR DMA transpose (no PE transposes).
B, S, D, H = 2, 2048, 1024, 16
DH = D // H
NC = 8
T = B * S                  # 4096 tokens
NCHUNK = T // 512          # 8 token chunks of 512
KCH = D // 128             # 8 contraction chunks
NJ = S // 128              # 16 key tiles per batch
QC = S // 512              # 4 query chunks per batch
NTT = T // 128             # 32 token tiles of 128 (v.T groups)

_CACHE = {}


def _build():
    if "nc" in _CACHE:
        return _CACHE["nc"]

    nc = bacc.Bacc("TRN2", target_bir_lowering=False, debug=False,
                   enable_asserts=True, num_devices=NC)

    xT = nc.dram_tensor("xT", [D, T], BF16, kind="ExternalInput").ap()
    wq = nc.dram_tensor("wq", [D, 128], BF16, kind="ExternalInput").ap()
    wk = nc.dram_tensor("wk", [D, 128], BF16, kind="ExternalInput").ap()
    wv = nc.dram_tensor("wv", [D, 128], BF16, kind="ExternalInput").ap()
    wo = nc.dram_tensor("wo", [128, D], BF16, kind="ExternalInput").ap()
    bq = nc.dram_tensor("bq", [128, 1], F32, kind="ExternalInput").ap()
    outT = nc.dram_tensor("outT", [D, T], BF16, kind="ExternalOutput").ap()

    with tile.TileContext(nc) as tc:
        with (
            tc.tile_pool(name="wpool", bufs=1) as wpool,
            tc.tile_pool(name="xin", bufs=8) as xin_pool,
            tc.tile_pool(name="epool", bufs=4) as epool,
            tc.tile_pool(name="vstg", bufs=2) as vstg_pool,
            tc.tile_pool(name="vgt", bufs=3) as vgt_pool,
            tc.tile_pool(name="onp", bufs=3) as on_pool,
            tc.tile_pool(name="npool", bufs=2) as npool,
            tc.tile_pool(name="ostage", bufs=3) as ostage_pool,
            # PSUM: spool 2x[128,1024]f32 (scores), opool 1x[65,1024]
            # (o accum), ppool 2x[128,512] (projections / out-projection)
            tc.tile_pool(name="spool", bufs=2, space="PSUM") as spool,
            tc.tile_pool(name="opool", bufs=1, space="PSUM") as opool,
            tc.tile_pool(name="ppool", bufs=2, space="PSUM") as ppool,
        ):
            # ---- persistent weights / activations ----
            wq_sb = wpool.tile([128, D], BF16, name="wq_sb")
            wk_sb = wpool.tile([128, D], BF16, name="wk_sb")
            wv_sb = wpool.tile([128, D], BF16, name="wv_sb")
            wo_sb = wpool.tile([128, D], BF16, name="wo_sb")
            bq_sb = wpool.tile([128, 1], F32, name="bq_sb")
            nc.sync.dma_start(wq_sb[:], wq.rearrange("(k p) f -> p k f", p=128))
            nc.sync.dma_start(wk_sb[:], wk.rearrange("(k p) f -> p k f", p=128))
            nc.sync.dma_start(wv_sb[:], wv.rearrange("(k p) f -> p k f", p=128))
            nc.sync.dma_start(wo_sb[:], wo[:])
            nc.sync.dma_start(bq_sb[:], bq[:])

            qn_all = wpool.tile([128, T], BF16, name="qn_all")
            kn_all = wpool.tile([128, T], BF16, name="kn_all")
            on = [wpool.tile([128, 512], BF16, name=f"on{n}")
                  for n in range(NCHUNK)]
            # v.T groups: one per 128-token tile, [tok, h0(64)|1|h1(64)|1]
            vg = [wpool.tile([128, 130], BF16, name=f"vg{J}")
                  for J in range(NTT)]

            # input chunks: xp[c][:, k*512 + t] = x.T[k*128+p, c*512+t]
            xp = [xin_pool.tile([128, KCH * 512], BF16, tag="xp",
                                name=f"xp{c}") for c in range(NCHUNK)]

            for c in range(NCHUNK):
                for k in range(KCH):
                    nc.sync.dma_start(xp[c][:, ts(k, 512)],
                                      xT[ts(k, 128), ts(c, 512)])

            # Warm the ACT exp table while the prologue runs.
            dummy = wpool.tile([1, 2], F32, name="dummy")
            nc.vector.memset(dummy[:], 0.0)
            nc.scalar.activation(dummy[:], dummy[:], EXP)

            # ones columns of the v.T groups (cols 64 and 129), set once
            for J in range(NTT):
                nc.vector.memset(vg[J][:], 1.0)

            # ---- emission helpers ----
            proj_state = {}

            def emit_kproj(c, k0, k1):
                if k0 == 0:
                    proj_state[("k", c)] = ppool.tile([128, 512], F32,
                                                      tag="P", name=f"kps{c}")
                ps = proj_state[("k", c)]
                for k in range(k0, k1):
                    nc.tensor.matmul(ps[:, 0:512], wk_sb[:, ts(k, 128)],
                                     xp[c][:, ts(k, 512)], start=(k == 0),
                                     stop=(k == KCH - 1))
                if k1 == KCH:
                    nc.vector.tensor_copy(kn_all[:, ts(c, 512)], ps[:, 0:512])

            def emit_qproj(c, k0, k1):
                if k0 == 0:
                    proj_state[("q", c)] = ppool.tile([128, 512], F32,
                                                      tag="P", name=f"qps{c}")
                ps = proj_state[("q", c)]
                for k in range(k0, k1):
                    nc.tensor.matmul(ps[:, 0:512], wq_sb[:, ts(k, 128)],
                                     xp[c][:, ts(k, 512)], start=(k == 0),
                                     stop=(k == KCH - 1))
                if k1 == KCH:
                    nc.vector.tensor_scalar_add(qn_all[:, ts(c, 512)],
                                                ps[:, 0:512], bq_sb[:])

            def emit_vt(J):
                # v.T for token tile J directly: out[tok, vfeat] =
                # sum_k x_slice[feat,tok].T @ WvT[feat,vfeat]
                c, tt = J // 4, J % 4
                ps = ppool.tile([128, 512], F32, tag="P", name=f"vtps{J}")
                for k in range(KCH):
                    nc.tensor.matmul(
                        ps[:, 0:128],
                        xp[c][:, k * 512 + tt * 128: k * 512 + (tt + 1) * 128],
                        wv_sb[:, ts(k, 128)], start=(k == 0),
                        stop=(k == KCH - 1))
                nc.vector.tensor_copy(
                    vg[J].rearrange("p (h c2) -> p h c2", h=2)[:, :, 0:64],
                    ps[:, 0:128].rearrange("p (h c2) -> p h c2", h=2))

            def emit_norm(n, o_ps, use_scalar=False):
                # critical path first: sums -> recip -> broadcast -> mult
                sums_sb = npool.tile([1, 1024], F32, tag="sums",
                                     name=f"sums{n}")
                nc.vector.tensor_copy(sums_sb[:], o_ps[64:65, :])
                r_sb = npool.tile([1, 1024], F32, tag="r", name=f"r{n}")
                nc.vector.reciprocal_approx_fast(r_sb[:], sums_sb[0:1, :])
                oc = npool.tile([64, 1024], F32, tag="oc", name=f"oc{n}")
                if use_scalar:
                    nc.scalar.copy(oc[:], o_ps[0:64, :])
                else:
                    nc.vector.tensor_copy(oc[:], o_ps[0:64, :])
                for hh in range(2):
                    hs = slice(hh * 64, (hh + 1) * 64)
                    rb = npool.tile([64, 512], F32, tag=f"rb{hh}",
                                    name=f"rb{n}_{hh}")
                    nc.gpsimd.partition_broadcast(
                        rb[:], r_sb[0:1, ts(hh, 512)])
                    nc.vector.tensor_tensor(
                        out=on[n][hs, :], in0=oc[0:64, ts(hh, 512)],
                        in1=rb[:], op=mybir.AluOpType.mult)

            def emit_outproj_m(n, m, use_scalar=False):
                op_ps = ppool.tile([128, 512], F32, tag="P",
                                   name=f"opps{n}_{m}")
                nc.tensor.matmul(op_ps[:, 0:512], wo_sb[:, ts(m, 128)],
                                 on[n][:], start=True, stop=True)
                ost = ostage_pool.tile([128, 512], BF16, tag="ost",
                                       name=f"ost{n}_{m}")
                if use_scalar:
                    nc.scalar.copy(ost[:], op_ps[:, 0:512])
                else:
                    nc.vector.tensor_copy(ost[:], op_ps[:, 0:512])
                nc.sync.dma_start(outT[ts(m, 128), ts(n, 512)], ost[:])

            # ---- deferred-work heap: (deadline_slot, seq, cost_ns, fn) ----
            seq = itertools.count()
            work = []

            def push(deadline, cost, fn):
                heapq.heappush(work, (deadline, next(seq), cost, fn))

            BUDGET = 650.0

            def pop_work(slot):
                spent = 0.0
                while work and (work[0][0] <= slot + 1 or spent < BUDGET):
                    _, _, cost, fn = heapq.heappop(work)
                    fn()
                    spent += cost

            # ---- prologue: just enough for attention slot 0 ----
            emit_kproj(0, 0, KCH)
            emit_qproj(0, 0, KCH)
            emit_vt(0)
            emit_vt(1)

            # b0 keys: chunk c needed by scores slot 4c
            for c in range(1, QC):
                push(4 * c - 3, 3400.0, (lambda c=c: emit_kproj(c, 0, KCH)))
            # b1 keys: needed from slot 64 on
            for c in range(QC, NCHUNK):
                push(64 + 4 * (c - QC) - 3, 3400.0,
                     (lambda c=c: emit_kproj(c, 0, KCH)))
            # v.T tiles: b0 tile J by o-acc slot J+1
            for J in range(2, NTT):
                if J < 16:
                    dl = J - 2
                else:
                    dl = 63 + (J - 16)
                push(dl, 1000.0, (lambda J=J: emit_vt(J)))
            # queries: chunk c by slot 16c
            for c in range(1, NCHUNK):
                push(16 * c - 5, 3400.0, (lambda c=c: emit_qproj(c, 0, KCH)))

            # ---- main attention loop ----
            for b in range(B):
                for qc in range(QC):
                    n = b * QC + qc
                    o_ps = opool.tile([65, 1024], F32, tag="O",
                                      name=f"ops{n}")
                    e_prev = None
                    for j in range(NJ):
                        slot = 16 * n + j
                        s_ps = spool.tile([128, 1024], F32, tag="S",
                                          name=f"sps{n}_{j}")
                        kblk = kn_all[:, b * S + j * 128: b * S + (j + 1) * 128]
                        for hh in range(2):
                            hs = slice(hh * 64, (hh + 1) * 64)
                            nc.tensor.matmul(
                                s_ps[:, ts(hh, 512)], kblk[hs, :],
                                qn_all[hs, ts(n, 512)], start=True, stop=True)
                        e_sb = epool.tile([128, 1024], BF16, tag="e",
                                          name=f"e{n}_{j}")
                        nc.scalar.activation(e_sb[:], s_ps[:], EXP)
                        if j >= 1:
                            for hh in range(2):
                                nc.tensor.matmul(
                                    o_ps[0:65, ts(hh, 512)],
                                    vg[b * NJ + j - 1][:,
                                                       hh * 65:(hh + 1) * 65],
                                    e_prev[:, ts(hh, 512)],
                                    start=(j - 1 == 0), stop=False)
                        e_prev = e_sb
                        pop_work(slot)
                    for hh in range(2):
                        nc.tensor.matmul(
                            o_ps[0:65, ts(hh, 512)],
                            vg[b * NJ + NJ - 1][:, hh * 65:(hh + 1) * 65],
                            e_prev[:, ts(hh, 512)], start=False, stop=True)
                    last = (n == NCHUNK - 1)
                    emit_norm(n, o_ps, use_scalar=last)
                    for m in range(KCH):
                        push(16 * (n + 1) + 4 + m, 400.0,
                             (lambda n=n, m=m, last=last:
                              emit_outproj_m(n, m, use_scalar=(last and
                                                               m % 2 == 1))))

            # drain whatever is left (last chunk's out-projection)
            while work:
                _, _, _, fn = heapq.heappop(work)
                fn()

    nc.compile()
    _CACHE["nc"] = nc
    return nc


def _prep_in_maps(inputs):
    import ml_dtypes
    bf16 = ml_dtypes.bfloat16
    x, Wq, bq = inputs["x"], inputs["Wq"], inputs["bq"]
    Wk, Wv, Wo = inputs["Wk"], inputs["Wv"], inputs["Wo"]
    xT = np.ascontiguousarray(x.reshape(T, D).T).astype(bf16)
    scale = np.float32(1.0 / np.sqrt(DH))
    in_maps = []
    for c in range(NC):
        sl = slice(128 * c, 128 * (c + 1))
        in_maps.append({
            "xT": xT,
            "wq": np.ascontiguousarray((scale * Wq[sl, :]).T).astype(bf16),
            "wk": np.ascontiguousarray(Wk[sl, :].T).astype(bf16),
            "wv": np.ascontiguousarray(Wv[sl, :].T).astype(bf16),
            "wo": np.ascontiguousarray(Wo[:, sl].T).astype(bf16),
            "bq": np.ascontiguousarray((scale * bq[sl])[:, None]),
        })
    return in_maps


def kernel(x, Wq, bq, Wk, bk, Wv, bv, Wo, bo):
    x = np.asarray(x, np.float32)
    Wq, bq = np.asarray(Wq, np.float32), np.asarray(bq, np.float32)
    Wk = np.asarray(Wk, np.float32)
    Wv, bv = np.asarray(Wv, np.float32), np.asarray(bv, np.float32)
    Wo, bo = np.asarray(Wo, np.float32), np.asarray(bo, np.float32)

    nc = _build()
    in_maps = _prep_in_maps({"x": x, "Wq": Wq, "bq": bq, "Wk": Wk,
                             "Wv": Wv, "Wo": Wo})
    res = bass_utils.run_bass_kernel_spmd(nc, in_maps, core_ids=list(range(NC)))

    acc = np.zeros((D, T), np.float64)
    for c in range(NC):
        acc += np.asarray(res.results[c]["outT"], np.float64)
    # v-bias folds through softmax (rows sum to 1): + bv @ Wo.T; plus bo.
    const = bo.astype(np.float64) + bv.astype(np.float64) @ Wo.T.astype(np.float64)
    out = acc.T + const[None, :]
    return out.astype(np.float32).reshape(B, S, D)


# revision 17
# speedup vs baseline: 1.1950x; 1.0026x over previous
import sys

if "/opt/trn_rl_repo" not in sys.path:
    sys.path.insert(0, "/opt/trn_rl_repo")

import heapq
import itertools

import numpy as np

import concourse.bacc as bacc
import concourse.tile as tile
from concourse import bass_utils, mybir
from concourse.bass import ts

F32 = mybir.dt.float32
BF16 = mybir.dt.bfloat16
EXP = mybir.ActivationFunctionType.Exp


# nn_MultiHeadedAttention: B=2, S=2048, D=1024, H=16, DH=64.
# 16 heads over 8 cores (2 heads/core = 128 features). QKV column-parallel,
# out-projection row-parallel, host sums the 8 partial outputs.
#
# K-bias is dropped entirely: softmax(q.(k+bk)) == softmax(q.k + const_per_q).
# V-bias folds through softmax on the host (rows of p sum to 1).
# V transposed to [tok, vfeat] via XBAR DMA transpose (no PE transposes).
B, S, D, H = 2, 2048, 1024, 16
DH = D // H
NC = 8
T = B * S                  # 4096 tokens
NCHUNK = T // 512          # 8 token chunks of 512
KCH = D // 128             # 8 contraction chunks
NJ = S // 128              # 16 key tiles per batch
QC = S // 512              # 4 query chunks per batch
NTT = T // 128             # 32 token tiles of 128 (v.T groups)

_CACHE = {}


def _build():
    if "nc" in _CACHE:
        return _CACHE["nc"]

    nc = bacc.Bacc("TRN2", target_bir_lowering=False, debug=False,
                   enable_asserts=True, num_devices=NC)

    xT = nc.dram_tensor("xT", [D, T], BF16, kind="ExternalInput").ap()
    wq = nc.dram_tensor("wq", [D, 128], BF16, kind="ExternalInput").ap()
    wk = nc.dram_tensor("wk", [D, 128], BF16, kind="ExternalInput").ap()
    wv = nc.dram_tensor("wv", [D, 128], BF16, kind="ExternalInput").ap()
    wo = nc.dram_tensor("wo", [128, D], BF16, kind="ExternalInput").ap()
    bq = nc.dram_tensor("bq", [128, 1], F32, kind="ExternalInput").ap()
    outT = nc.dram_tensor("outT", [D, T], BF16, kind="ExternalOutput").ap()

    with tile.TileContext(nc) as tc:
        with (
            tc.tile_pool(name="wpool", bufs=1) as wpool,
            tc.tile_pool(name="xin", bufs=8) as xin_pool,
            tc.tile_pool(name="epool", bufs=4) as epool,
            tc.tile_pool(name="vstg", bufs=2) as vstg_pool,
            tc.tile_pool(name="vgt", bufs=3) as vgt_pool,
            tc.tile_pool(name="onp", bufs=3) as on_pool,
            tc.tile_pool(name="npool", bufs=2) as npool,
            tc.tile_pool(name="ostage", bufs=3) as ostage_pool,
            # PSUM: spool 2x[128,1024]f32 (scores), opool 1x[65,1024]
            # (o accum), ppool 2x[128,512] (projections / out-projection)
            tc.tile_pool(name="spool", bufs=2, space="PSUM") as spool,
            tc.tile_pool(name="opool", bufs=1, space="PSUM") as opool,
            tc.tile_pool(name="ppool", bufs=2, space="PSUM") as ppool,
        ):
            # ---- persistent weights / activations ----
            wq_sb = wpool.tile([128, D], BF16, name="wq_sb")
            wk_sb = wpool.tile([128, D], BF16, name="wk_sb")
            wv_sb = wpool.tile([128, D], BF16, name="wv_sb")
            wo_sb = wpool.tile([128, D], BF16, name="wo_sb")
            bq_sb = wpool.tile([128, 1], F32, name="bq_sb")
            # K-proj weights and x chunk 0 first: they gate the prologue
            nc.sync.dma_start(wk_sb[:], wk.rearrange("(k p) f -> p k f", p=128))

            qn_all = wpool.tile([128, T], BF16, name="qn_all")
            kn_all = wpool.tile([128, T], BF16, name="kn_all")
            on = [wpool.tile([128, 512], BF16, name=f"on{n}")
                  for n in range(NCHUNK)]
            # v.T groups: one per 128-token tile, [tok, h0(64)|1|h1(64)|1]
            vg = [wpool.tile([128, 130], BF16, name=f"vg{J}")
                  for J in range(NTT)]

            # input chunks: xp[c][:, k*512 + t] = x.T[k*128+p, c*512+t]
            xp = [xin_pool.tile([128, KCH * 512], BF16, tag="xp",
                                name=f"xp{c}") for c in range(NCHUNK)]

            def xload(c):
                for k in range(KCH):
                    nc.sync.dma_start(xp[c][:, ts(k, 512)],
                                      xT[ts(k, 128), ts(c, 512)])

            xload(0)
            nc.sync.dma_start(wq_sb[:], wq.rearrange("(k p) f -> p k f", p=128))
            nc.sync.dma_start(bq_sb[:], bq[:])
            nc.sync.dma_start(wv_sb[:], wv.rearrange("(k p) f -> p k f", p=128))
            xload(1)
            nc.sync.dma_start(wo_sb[:], wo[:])
            for c in range(2, NCHUNK):
                xload(c)

            # Warm the ACT exp table while the prologue runs.
            dummy = wpool.tile([1, 2], F32, name="dummy")
            nc.vector.memset(dummy[:], 0.0)
            nc.scalar.activation(dummy[:], dummy[:], EXP)

            # ones columns of the v.T groups (cols 64 and 129), set once
            for J in range(NTT):
                nc.vector.memset(vg[J][:], 1.0)

            # ---- emission helpers ----
            proj_state = {}

            def emit_kproj(c, k0, k1):
                if k0 == 0:
                    proj_state[("k", c)] = ppool.tile([128, 512], F32,
                                                      tag="P", name=f"kps{c}")
                ps = proj_state[("k", c)]
                for k in range(k0, k1):
                    nc.tensor.matmul(ps[:, 0:512], wk_sb[:, ts(k, 128)],
                                     xp[c][:, ts(k, 512)], start=(k == 0),
                                     stop=(k == KCH - 1))
                if k1 == KCH:
                    nc.vector.tensor_copy(kn_all[:, ts(c, 512)], ps[:, 0:512])

            def emit_qproj(c, k0, k1):
                if k0 == 0:
                    proj_state[("q", c)] = ppool.tile([128, 512], F32,
                                                      tag="P", name=f"qps{c}")
                ps = proj_state[("q", c)]
                for k in range(k0, k1):
                    nc.tensor.matmul(ps[:, 0:512], wq_sb[:, ts(k, 128)],
                                     xp[c][:, ts(k, 512)], start=(k == 0),
                                     stop=(k == KCH - 1))
                if k1 == KCH:
                    nc.vector.tensor_scalar_add(qn_all[:, ts(c, 512)],
                                                ps[:, 0:512], bq_sb[:])

            def emit_vt(J):
                # v.T for token tile J directly: out[tok, vfeat] =
                # sum_k x_slice[feat,tok].T @ WvT[feat,vfeat]
                c, tt = J // 4, J % 4
                ps = ppool.tile([128, 512], F32, tag="P", name=f"vtps{J}")
                for k in range(KCH):
                    nc.tensor.matmul(
                        ps[:, 0:128],
                        xp[c][:, k * 512 + tt * 128: k * 512 + (tt + 1) * 128],
                        wv_sb[:, ts(k, 128)], start=(k == 0),
                        stop=(k == KCH - 1))
                nc.vector.tensor_copy(
                    vg[J].rearrange("p (h c2) -> p h c2", h=2)[:, :, 0:64],
                    ps[:, 0:128].rearrange("p (h c2) -> p h c2", h=2))

            def emit_norm(n, o_ps, use_scalar=False):
                # critical path first: sums -> recip -> broadcast -> mult
                sums_sb = npool.tile([1, 1024], F32, tag="sums",
                                     name=f"sums{n}")
                nc.vector.tensor_copy(sums_sb[:], o_ps[64:65, :])
                r_sb = npool.tile([1, 1024], F32, tag="r", name=f"r{n}")
                nc.vector.reciprocal_approx_fast(r_sb[:], sums_sb[0:1, :])
                oc = npool.tile([64, 1024], F32, tag="oc", name=f"oc{n}")
                if use_scalar:
                    nc.scalar.copy(oc[:], o_ps[0:64, :])
                else:
                    nc.vector.tensor_copy(oc[:], o_ps[0:64, :])
                for hh in range(2):
                    hs = slice(hh * 64, (hh + 1) * 64)
                    rb = npool.tile([64, 512], F32, tag=f"rb{hh}",
                                    name=f"rb{n}_{hh}")
                    nc.gpsimd.partition_broadcast(
                        rb[:], r_sb[0:1, ts(hh, 512)])
                    nc.vector.tensor_tensor(
                        out=on[n][hs, :], in0=oc[0:64, ts(hh, 512)],
                        in1=rb[:], op=mybir.AluOpType.mult)

            def emit_outproj_m(n, m, use_scalar=False):
                op_ps = ppool.tile([128, 512], F32, tag="P",
                                   name=f"opps{n}_{m}")
                nc.tensor.matmul(op_ps[:, 0:512], wo_sb[:, ts(m, 128)],
                                 on[n][:], start=True, stop=True)
                ost = ostage_pool.tile([128, 512], BF16, tag="ost",
                                       name=f"ost{n}_{m}")
                if use_scalar:
                    nc.scalar.copy(ost[:], op_ps[:, 0:512])
                else:
                    nc.vector.tensor_copy(ost[:], op_ps[:, 0:512])
                nc.sync.dma_start(outT[ts(m, 128), ts(n, 512)], ost[:])

            # ---- deferred-work heap: (deadline_slot, seq, cost_ns, fn) ----
            seq = itertools.count()
            work = []

            def push(deadline, cost, fn):
                heapq.heappush(work, (deadline, next(seq), cost, fn))

            BUDGET = 650.0

            def pop_work(slot):
                spent = 0.0
                while work and (work[0][0] <= slot + 1 or spent < BUDGET):
                    _, _, cost, fn = heapq.heappop(work)
                    fn()
                    spent += cost

            # ---- prologue: just enough for attention slot 0 ----
            emit_kproj(0, 0, KCH)
            emit_qproj(0, 0, KCH)
            emit_vt(0)
            emit_vt(1)

            # b0 keys: chunk c needed by scores slot 4c
            for c in range(1, QC):
                push(4 * c - 3, 3400.0, (lambda c=c: emit_kproj(c, 0, KCH)))
            # b1 keys: needed from slot 64 on, spread through b0 blocks 3-4
            for c in range(QC, NCHUNK):
                push(52 + 3 * (c - QC), 3400.0,
                     (lambda c=c: emit_kproj(c, 0, KCH)))
            # v.T tiles: b0 tile J by o-acc slot J+1, b1 spread over blocks 1-3
            for J in range(2, NTT):
                if J < 16:
                    dl = J - 2
                else:
                    dl = 20 + 2 * (J - 16)
                push(dl, 1000.0, (lambda J=J: emit_vt(J)))
            # queries: chunk c by slot 16c
            for c in range(1, NCHUNK):
                push(16 * c - 5, 3400.0, (lambda c=c: emit_qproj(c, 0, KCH)))

            # ---- main attention loop ----
            for b in range(B):
                for qc in range(QC):
                    n = b * QC + qc
                    o_ps = opool.tile([65, 1024], F32, tag="O",
                                      name=f"ops{n}")
                    e_prev = None
                    for j in range(NJ):
                        slot = 16 * n + j
                        s_ps = spool.tile([128, 1024], F32, tag="S",
                                          name=f"sps{n}_{j}")
                        kblk = kn_all[:, b * S + j * 128: b * S + (j + 1) * 128]
                        for hh in range(2):
                            hs = slice(hh * 64, (hh + 1) * 64)
                            nc.tensor.matmul(
                                s_ps[:, ts(hh, 512)], kblk[hs, :],
                                qn_all[hs, ts(n, 512)], start=True, stop=True)
                        e_sb = epool.tile([128, 1024], BF16, tag="e",
                                          name=f"e{n}_{j}")
                        nc.scalar.activation(e_sb[:], s_ps[:], EXP)
                        if j >= 1:
                            for hh in range(2):
                                nc.tensor.matmul(
                                    o_ps[0:65, ts(hh, 512)],
                                    vg[b * NJ + j - 1][:,
                                                       hh * 65:(hh + 1) * 65],
                                    e_prev[:, ts(hh, 512)],
                                    start=(j - 1 == 0), stop=False)
                        e_prev = e_sb
                        pop_work(slot)
                    for hh in range(2):
                        nc.tensor.matmul(
                            o_ps[0:65, ts(hh, 512)],
                            vg[b * NJ + NJ - 1][:, hh * 65:(hh + 1) * 65],
                            e_prev[:, ts(hh, 512)], start=False, stop=True)
                    last = (n == NCHUNK - 1)
                    emit_norm(n, o_ps, use_scalar=last)
                    for m in range(KCH):
                        push(16 * (n + 1) + 4 + m, 400.0,
                             (lambda n=n, m=m, last=last:
                              emit_outproj_m(n, m, use_scalar=(last and
                                                               m % 2 == 1))))

            # drain whatever is left (last chunk's out-projection)
            while work:
                _, _, _, fn = heapq.heappop(work)
                fn()

    nc.compile()
    _CACHE["nc"] = nc
    return nc


def _prep_in_maps(inputs):
    import ml_dtypes
    bf16 = ml_dtypes.bfloat16
    x, Wq, bq = inputs["x"], inputs["Wq"], inputs["bq"]
    Wk, Wv, Wo = inputs["Wk"], inputs["Wv"], inputs["Wo"]
    xT = np.ascontiguousarray(x.reshape(T, D).T).astype(bf16)
    scale = np.float32(1.0 / np.sqrt(DH))
    in_maps = []
    for c in range(NC):
        sl = slice(128 * c, 128 * (c + 1))
        in_maps.append({
            "xT": xT,
            "wq": np.ascontiguousarray((scale * Wq[sl, :]).T).astype(bf16),
            "wk": np.ascontiguousarray(Wk[sl, :].T).astype(bf16),
            "wv": np.ascontiguousarray(Wv[sl, :].T).astype(bf16),
            "wo": np.ascontiguousarray(Wo[:, sl].T).astype(bf16),
            "bq": np.ascontiguousarray((scale * bq[sl])[:, None]),
        })
    return in_maps


def kernel(x, Wq, bq, Wk, bk, Wv, bv, Wo, bo):
    x = np.asarray(x, np.float32)
    Wq, bq = np.asarray(Wq, np.float32), np.asarray(bq, np.float32)
    Wk = np.asarray(Wk, np.float32)
    Wv, bv = np.asarray(Wv, np.float32), np.asarray(bv, np.float32)
    Wo, bo = np.asarray(Wo, np.float32), np.asarray(bo, np.float32)

    nc = _build()
    in_maps = _prep_in_maps({"x": x, "Wq": Wq, "bq": bq, "Wk": Wk,
                             "Wv": Wv, "Wo": Wo})
    res = bass_utils.run_bass_kernel_spmd(nc, in_maps, core_ids=list(range(NC)))

    acc = np.zeros((D, T), np.float64)
    for c in range(NC):
        acc += np.asarray(res.results[c]["outT"], np.float64)
    # v-bias folds through softmax (rows sum to 1): + bv @ Wo.T; plus bo.
    const = bo.astype(np.float64) + bv.astype(np.float64) @ Wo.T.astype(np.float64)
    out = acc.T + const[None, :]
    return out.astype(np.float32).reshape(B, S, D)
